# revision 22
# baseline (speedup 1.0000x reference)
"""Trainium2 Bass kernel for the 16-level ternary (Haar-style) wavelet
transform of f (len 3^16) with row-orthonormalized 3x3 Phi matrices.

Strategy (v3, fp16 + PE diag-matmuls):
  - Host: QR-orthonormalize the 3x3 Phi blocks; convert f to fp16; build
    per-level diagonal stationary matrices diag(Phi[lvl, r, j]) in fp16.
  - Main SPMD kernel (8 cores): f split into contiguous chunks aligned to
    units of 3^7 = 2187 elements; each unit recurses levels 0..6 inside
    one SBUF partition.  All device data is fp16 (halves HBM traffic; the
    harness gate is rel_err < 2e-2, this measures ~1.4e-3).
  - Detail outputs d1/d2 of levels 0..2 run on the idle PE engine: three
    diagonal-stationary matmuls per output accumulate c0*X0+c1*X1+c2*X2
    in PSUM (fp32) while reading the interleaved signal directly with
    stride-3 moving APs.  ACT evacuates PSUM -> SBUF fp16.
  - The average chain (recursion) stays elementwise: ACT tensor-scalar
    mult + two DVE scalar_tensor_tensor macs, strided reads.
  - Levels 3..6 are batched elementwise (tiny).  Levels 7..15 (19683
    elems = 0.05% of work) run on the host in fp32; no collective.
"""

import sys

for _p in ("/opt/trn_rl_repo",):
    if _p not in sys.path:
        sys.path.append(_p)

import numpy as np

import concourse.bass as bass
import concourse.mybir as mybir
import concourse.tile as tile

F16 = mybir.dt.float16
F32 = mybir.dt.float32
MULT = mybir.AluOpType.mult
ADD = mybir.AluOpType.add

NL = 16                   # total levels
LK = 7                    # levels computed on device (0..6)
UNIT = 3 ** LK            # 2187 input elems per unit
NUNITS = 3 ** (NL - LK)   # 19683 units overall
NCORES = 8
UPP = 4                   # units per partition per tile
T = 5                     # tiles per core
PAD_UNITS = T * 128 * UPP  # 2560 padded units per core
NPE = 3                   # levels using the PE path (0..NPE-1)

# contiguous unit ranges per core (2461 x7 + 2456)
_base = [0]
for _k in range(NCORES):
    _base.append(_base[-1] + (2461 if _k < 7 else NUNITS - 7 * 2461))
CORE_U0 = _base[:-1]
CORE_UN = [_base[k + 1] - _base[k] for k in range(NCORES)]

# main-kernel output layout (per core, in elements)
OFF_D1 = []
OFF_D2 = []
_off = 0
for _i in range(LK):
    _w = 3 ** (6 - _i)
    OFF_D1.append(_off)
    OFF_D2.append(_off + PAD_UNITS * _w)
    _off += 2 * PAD_UNITS * _w
OFF_F7 = _off
OUT_LEN = _off + PAD_UNITS


def _split_multi_waits(nc):
    """This walrus build rejects any instruction carrying >1 sync wait
    ("Too many sync wait commands").  Split extra waits onto single-wait
    NOPs inserted just before, on the same engine queue (queue order makes
    the semantics identical)."""
    ctr = [0]
    for fn in nc.m.functions:
        for bb in fn.blocks:
            new = []
            for inst in bb.instructions:
                si = inst.sync_info
                if si is not None and si.on_wait and len(si.on_wait) > 1:
                    waits = list(si.on_wait)
                    for w in waits[:-1]:
                        ctr[0] += 1
                        new.append(mybir.InstNoOp(
                            name=f"splitw_{ctr[0]}",
                            engine=inst.engine,
                            bass_nofuse=True,
                            sync_info=mybir.SyncInfo(on_wait=[w], on_update=[]),
                        ))
                    si.on_wait = [waits[-1]]
                new.append(inst)
            bb.instructions = new


def _avg(nc, av_dst, cur, Wo, phi_sb, lvl, mul_act=False):
    """av = Phi[lvl,0,:] . cur triples: mult + 2 DVE macs, strided.
    mul_act puts the first mult on ACT (engine balance)."""
    c = lambda k: phi_sb[:, lvl * 9 + k: lvl * 9 + k + 1]
    if mul_act:
        nc.scalar.mul(av_dst, cur[:, 0::3], c(0))
    else:
        nc.vector.tensor_scalar_mul(av_dst, cur[:, 0::3], c(0))
    nc.vector.scalar_tensor_tensor(av_dst, cur[:, 1::3], c(1), av_dst,
                                   MULT, ADD)
    nc.vector.scalar_tensor_tensor(av_dst, cur[:, 2::3], c(2), av_dst,
                                   MULT, ADD)


SUP = 1024  # PSUM super-tile columns (2 banks); matmuls in 512-col slices


def _details_pe(nc, psum_pool, diags, cur, Wo, lvl, dsb, out, dst_off, uw):
    """d1/d2 via diagonal-stationary matmuls accumulating in PSUM (512-col
    bank-aligned slices of a 2-bank super-tile), then one ACT evac copy
    (f32->f16 cast) per super-tile into dsb, DMA'd out immediately."""
    nsc = (Wo + SUP - 1) // SUP
    for si in range(nsc):
        s0 = si * SUP
        scols = min(SUP, Wo - s0)
        for half in range(2):
            acc = psum_pool.tile([128, SUP], F32, tag="acc",
                                 name=f"acc{lvl}_{si}_{half}")
            for k in range((scols + 511) // 512):
                c0 = s0 + k * 512
                cols = min(512, scols - k * 512)
                for j in range(3):
                    stat = diags[:, (lvl * 6 + half * 3 + j) * 128:
                                 (lvl * 6 + half * 3 + j + 1) * 128]
                    m0 = 3 * c0 + j
                    mv = cur[:, m0: m0 + 3 * (cols - 1) + 1: 3]
                    nc.tensor.matmul(acc[:, k * 512: k * 512 + cols], stat,
                                     mv, start=(j == 0), stop=(j == 2))
            sb = dsb[:, half * Wo + s0: half * Wo + s0 + scols]
            nc.scalar.copy(sb, acc[:, :scols])


def _details_elem(nc, cur, Wo, phi_sb, lvl, dsb):
    """d1/d2 elementwise (deep levels): ACT mult + DVE macs, strided."""
    c = lambda k: phi_sb[:, lvl * 9 + k: lvl * 9 + k + 1]
    for half in range(2):
        d = dsb[:, half * Wo:(half + 1) * Wo]
        nc.scalar.mul(d, cur[:, 0::3], c(3 * half + 3))
        nc.vector.scalar_tensor_tensor(d, cur[:, 1::3], c(3 * half + 4), d,
                                       MULT, ADD)
        nc.vector.scalar_tensor_tensor(d, cur[:, 2::3], c(3 * half + 5), d,
                                       MULT, ADD)


def build_main(nrep=1, in_bufs=3, a_bufs=2, d_bufs=2, ps_bufs=3,
               split_dma=False):
    nc = bass.Bass("TRN2", target_bir_lowering=False, debug=False,
                   num_devices=NCORES)
    x = nc.dram_tensor("x", [PAD_UNITS * UNIT], F16, kind="ExternalInput")
    phi = nc.dram_tensor("phi", [128, LK * 9], F32, kind="ExternalInput")
    dg = nc.dram_tensor("dg", [128, NPE * 6 * 128], F16,
                        kind="ExternalInput")
    out = nc.dram_tensor("out", [OUT_LEN], F16, kind="ExternalOutput")

    FW = UPP * UNIT  # 8748 elems per partition per tile

    with tile.TileContext(nc) as tc:
        with (
            tc.tile_pool(name="phi_p", bufs=1) as phi_pool,
            tc.tile_pool(name="in_p", bufs=in_bufs) as in_pool,
            tc.tile_pool(name="a_p", bufs=a_bufs) as a_pool,
            tc.tile_pool(name="d_p", bufs=d_bufs) as d_pool,
            tc.tile_pool(name="r_p", bufs=1) as r_pool,
            tc.tile_pool(name="ps", bufs=ps_bufs, space="PSUM") as psum_pool,
        ):
            def body():
                phi_sb = phi_pool.tile([128, LK * 9], F32, tag="phi",
                                       name="phi_sb")
                nc.sync.dma_start(phi_sb[:], phi[:])
                diags = phi_pool.tile([128, NPE * 6 * 128], F16, tag="dg",
                                      name="diags")
                nc.sync.dma_start(diags[:], dg[:])

                R = {lvl: r_pool.tile(
                        [128, T * UPP * 3 ** (7 - lvl)], F16,
                        tag=f"R{lvl}", name=f"R{lvl}")
                     for lvl in range(3, LK)}
                F7 = r_pool.tile([128, T * UPP], F16, tag="F7", name="F7")

                # ---- streamed levels 0..2, one [128, UPP*2187] tile each
                for t in range(T):
                    xt = in_pool.tile([128, FW], F16, tag="xt", name="xt")
                    src = bass.AP(x, t * 128 * FW, [[FW, 128], [1, FW]])
                    nc.sync.dma_start(xt[:], src)

                    # average chain first (recursion-critical)
                    curs = [xt[:]]
                    for lvl in range(3):
                        Wo = curs[-1].shape[-1] // 3
                        if lvl < 2:
                            av = a_pool.tile([128, Wo], F16, tag=f"a{lvl}",
                                             name=f"a{lvl}")
                            av_ap = av[:]
                        else:
                            av_ap = R[3][:, t * UPP * 81:(t + 1) * UPP * 81]
                        _avg(nc, av_ap, curs[-1], Wo, phi_sb, lvl)
                        curs.append(av_ap)

                    # details via PE + evac, one merged DMA out per level
                    for lvl in range(3):
                        cur = curs[lvl]
                        Wo = cur.shape[-1] // 3
                        w = 3 ** (6 - lvl)
                        dd = d_pool.tile([128, 2 * Wo], F16, tag=f"d{lvl}",
                                         name=f"d{lvl}")
                        uw = UPP * w
                        _details_pe(nc, psum_pool, diags, cur, Wo, lvl,
                                    dd[:], out, OFF_D1[lvl] + t * 128 * uw,
                                    uw)
                        dst = bass.AP(out, OFF_D1[lvl] + t * 128 * uw,
                                      [[uw, 128], [PAD_UNITS * w, 2],
                                       [1, uw]])
                        out_q = nc.scalar if split_dma else nc.sync
                        out_q.dma_start(
                            dst, dd[:].rearrange("p (j c) -> p j c", j=2))

                # ---- batched levels 3..6, elementwise
                for lvl in range(3, LK):
                    w = 3 ** (6 - lvl)
                    cur = R[lvl][:]
                    Wo = cur.shape[-1] // 3
                    av_ap = R[lvl + 1][:] if lvl < 6 else F7[:]
                    _avg(nc, av_ap, cur, Wo, phi_sb, lvl)
                    dd = d_pool.tile([128, 2 * Wo], F16, tag=f"db{lvl}",
                                     name=f"db{lvl}")
                    _details_elem(nc, cur, Wo, phi_sb, lvl, dd[:])
                    uw = UPP * w
                    for half, off in ((0, OFF_D1[lvl]), (1, OFF_D2[lvl])):
                        dst = bass.AP(out, off,
                                      [[uw, 128], [128 * uw, T], [1, uw]])
                        src3 = dd[:, half * Wo:(half + 1) * Wo].rearrange(
                            "p (t c) -> p t c", t=T)
                        nc.sync.dma_start(dst, src3)

                # f7 slab out: value for unit g = t*512 + p*4 + j
                dstf = bass.AP(out, OFF_F7,
                               [[UPP, 128], [128 * UPP, T], [1, UPP]])
                nc.sync.dma_start(dstf, F7[:].rearrange("p (t j) -> p t j",
                                                        t=T))

            if nrep == 1:
                body()
            else:
                with tc.For_i(0, nrep, 1):
                    body()

    return nc


def _phi_from_inputs(Phi_P: np.ndarray) -> np.ndarray:
    Q = np.stack([np.linalg.qr(Phi_P[i].T.astype(np.float32))[0]
                  for i in range(Phi_P.shape[0])])
    return np.transpose(Q, (0, 2, 1)).astype(np.float32)


def _diags_from_phi(Phi: np.ndarray) -> np.ndarray:
    """[128, NPE*6*128] fp16: diag(Phi[lvl, 1+half, j]) blocks."""
    dgv = np.zeros((128, NPE * 6 * 128), np.float16)
    idx = np.arange(128)
    for lvl in range(NPE):
        for half in range(2):
            for j in range(3):
                col = (lvl * 6 + half * 3 + j) * 128
                dgv[idx, col + idx] = np.float16(Phi[lvl, 1 + half, j])
    return dgv


_CACHE = {}


def _make_runner(nc):
    """Compile-once SPMD runner (the core of bass2jax.run_bass_via_pjrt,
    kept so repeat kernel() calls skip re-tracing/compiling).  No output
    donation: this kernel writes every element of its outputs."""
    import jax
    from jax.sharding import Mesh, PartitionSpec, NamedSharding
    from jax.experimental.shard_map import shard_map
    from concourse.bass2jax import (
        _bass_exec_p, partition_id_tensor, install_neuronx_cc_hook,
    )

    install_neuronx_cc_hook()
    pname = nc.partition_id_tensor.name if nc.partition_id_tensor else None
    in_names, out_names, out_avals, zero_outs = [], [], [], []
    for alloc in nc.m.functions[0].allocations:
        if not isinstance(alloc, mybir.MemoryLocationSet):
            continue
        name = alloc.memorylocations[0].name
        if alloc.kind == "ExternalInput":
            if name != pname:
                in_names.append(name)
        elif alloc.kind == "ExternalOutput":
            shape = tuple(alloc.tensor_shape)
            dtype = mybir.dt.np(alloc.dtype)
            out_names.append(name)
            out_avals.append(jax.core.ShapedArray(shape, dtype))
            zero_outs.append(np.zeros(shape, dtype))
    n_params = len(in_names)
    all_in = list(in_names) + list(out_names)
    if pname is not None:
        all_in.append(pname)

    def _body(*args):
        operands = list(args)
        if pname is not None:
            operands.append(partition_id_tensor())
        return tuple(_bass_exec_p.bind(
            *operands,
            out_avals=tuple(out_avals),
            in_names=tuple(all_in),
            out_names=tuple(out_names),
            lowering_input_output_aliases=(),
            sim_require_finite=True,
            sim_require_nnan=True,
            nc=nc,
        ))

    devices = jax.devices()[:NCORES]
    mesh = Mesh(np.asarray(devices), ("core",))
    spec = PartitionSpec("core")
    run = jax.jit(
        shard_map(_body, mesh=mesh,
                  in_specs=(spec,) * (n_params + len(out_names)),
                  out_specs=(spec,) * len(out_names), check_rep=False),
        keep_unused=True,
    )
    sharding = NamedSharding(mesh, spec)
    zeros_dev = [
        jax.device_put(
            np.zeros((NCORES * z.shape[0], *z.shape[1:]), z.dtype), sharding)
        for z in zero_outs
    ]

    def execute(in_maps):
        concat = [
            np.concatenate([np.asarray(m[name]) for m in in_maps], axis=0)
            for name in in_names
        ]
        outs = run(*concat, *zeros_dev)
        return [
            {name: np.asarray(outs[i]).reshape(NCORES, *out_avals[i].shape)[c]
             for i, name in enumerate(out_names)}
            for c in range(NCORES)
        ]

    return execute


def kernel(f: np.ndarray, Phi_P: np.ndarray) -> np.ndarray:
    f = np.asarray(f, dtype=np.float32).ravel()
    Phi = _phi_from_inputs(np.asarray(Phi_P, dtype=np.float32))

    phi_all = np.broadcast_to(
        Phi[:LK].reshape(1, LK * 9), (128, LK * 9)).astype(np.float32).copy()
    dgv = _diags_from_phi(Phi)

    if "run" not in _CACHE:
        nc_main = build_main()
        _split_multi_waits(nc_main)
        _CACHE["run"] = _make_runner(nc_main)

    f16 = f.astype(np.float16)
    in_maps = []
    for k in range(NCORES):
        lo = CORE_U0[k] * UNIT
        n = CORE_UN[k] * UNIT
        xk = np.zeros(PAD_UNITS * UNIT, dtype=np.float16)
        xk[:n] = f16[lo:lo + n]
        in_maps.append({"x": xk, "phi": phi_all, "dg": dgv})

    results = _CACHE["run"](in_maps)

    f_hat = np.empty(3 ** NL, dtype=np.float32)
    f7g = np.empty(NUNITS, dtype=np.float32)
    for k in range(NCORES):
        ok = results[k]["out"]
        u0, un = CORE_U0[k], CORE_UN[k]
        for i in range(LK):
            w = 3 ** (6 - i)
            base = 3 ** (15 - i)
            f_hat[base + u0 * w: base + (u0 + un) * w] = \
                ok[OFF_D1[i]: OFF_D1[i] + un * w].astype(np.float32)
            f_hat[2 * base + u0 * w: 2 * base + (u0 + un) * w] = \
                ok[OFF_D2[i]: OFF_D2[i] + un * w].astype(np.float32)
        f7g[u0:u0 + un] = ok[OFF_F7: OFF_F7 + un].astype(np.float32)

    # levels 7..15 on the host (19683 elems, fp32)
    cur = f7g
    for i in range(LK, NL):
        fm = cur.reshape(-1, 3)
        base = 3 ** (15 - i)
        f_hat[base: 2 * base] = fm @ Phi[i, 1]
        f_hat[2 * base: 3 * base] = fm @ Phi[i, 2]
        cur = fm @ Phi[i, 0]
    f_hat[0] = cur[0]
    return f_hat
